# revision 46
# baseline (speedup 1.0000x reference)
"""Trainium2 Bass kernel for nn_Attention_org_45758581571643.

Reference computation (per batch b):
  x = emb[b] viewed as [S=T*N, C] (token-major)
  per head h: Q/K/V = x @ W{q,k,v}[h].T ; scores = Q K^T / sqrt(S)
  InstanceNorm over each [S,S] map, softmax over keys, ctx = probs @ V
  out = mean_h(ctx) @ Wo.T, reshaped to [B, T, C, N]

Sharding: 16 (batch, head) pairs over 8 cores -> core c handles batch c//2,
heads {2*(c%2), 2*(c%2)+1}. Head-mean and the Wo projection are linear, so each
core applies Wo to its own two-head partial sum and the host adds core pairs.

The device runs ONLY the two O(S^2) matmuls per head; everything that is
O(S*C^2) or smaller is folded on the host:
- gt[h] = (Wq[h]^T Wk[h])^T x^T is precomputed, so scores^T = x^T^T gt.
- vm[h] = [x Wv[h]^T Wo^T | 4*mask] is precomputed in bf16; the mask column
  zeroes the padded key rows and the 4 folds the mean over H=4 heads.
- The instance-norm statistics come from closed forms (sum(scores) = u^T G u
  with u = x^T 1; sum(scores^2) = <G, A G A> with A = x^T x); only the exp
  scale sigma = 1/sqrt(var_raw + S*eps) is shipped. Mean-centering is dropped
  (softmax is shift-invariant and |sigma*scores| < ~6), so exp fuses into the
  score matmul's PSUM->SBUF eviction with a single ACT pass.
- Softmax denominators ride along the ctx matmul for free: probs tiles are the
  stationary operand and vm is the moving operand, so column 256 of each ctx
  PSUM tile is 4*sum_t p[t,s]; DVE scales by its reciprocal per partition.

On-device layout: x lives as [C, S] (channel on partitions), probs as [t, s]
(keys on partitions), ctx as [s, d] (queries on partitions). The key axis is
zero-padded to 1664 = 13*128; padded rows have V'=0 and mask=0. Scores/exp are
emitted band-pair-major so each ctx chunk's exp dependencies complete
progressively; probs tiles are double-buffered so consecutive heads/reps
pipeline with no PE stalls.

Hardware-profile-driven tuning (NTFF steady-state rep: 79.2us -> 69.6us,
~99% of the 68.8us PE issue-rate floor; the contraction pass structure is
minimal for >=16-bit precision and fp8 DoubleRow fails the 2e-2 gate):
- xt/gt in fp16: fp32 LDWEIGHTS (189ns) exceeded the 163ns scores MM and
  serialized the weight path; 16-bit weights use FWL (97ns) and hide fully.
- Scores PSUM tiles are [128, 2, 512] f32 (2 banks, chunk stride = bank), so
  one ACT instruction evicts 784 columns: the 172-cycle per-instruction
  PSUM-access bubble made single-bank evictions the scores-phase governor.
- PSUM split 3x2-bank scores + 2x1-bank ctx (8 banks): 3 scores buffers let
  PE run two tiles ahead of ACT; the Tile scheduler fills the remaining ACT
  slack with ctx MMs of the other head.
- The 32-wide remainders (query chunk 12, key tile 12) are head-packed: both
  heads' MMs issue to PE column groups 0/32 and execute concurrently
  (~2x on those MMs). Col-packed outputs force h1's exp rows to partitions
  32:64, so a SBUF->SBUF DMA shifts them into a full-row tile off the
  critical path -- partial-row stationaries would disable background
  weight loading (~+200ns per chain). h1's share of query chunk 12 can't be
  added to h0's on-device (DVE ops can't cross partitions); it ships via the
  ot2 output and the host adds.
- ACT paces each scores phase (832ns/tile eviction vs 652ns of MMs), so the
  rep loop is software-pipelined: ctx chains emit as step generators and the
  scores loop weaves 2 chain MMs into each tile's eviction slack. ctx(h1) of
  rep r defers to weave into rep r+1's scores(h0) — it reads st1, which
  isn't rewritten until scores(h1) of r+1, so it crosses the rep boundary
  freely. This cut PE stall excess from 3.2us to ~0.8us (semaphore
  propagation at chain starts, ~100ns SEM_DELAY each).
"""

import os

# Recover gracefully if a previous run left a NeuronCore wedged; must be set
# before the runtime initializes.
os.environ.setdefault("NEURON_RT_RESET_CORES", "1")

import numpy as np
from contextlib import ExitStack

B, T, C, N, H = 4, 8, 256, 196, 4
S = T * N          # 1568
SP = 1664          # 13 * 128 (padded key/seq length)
NT = SP // 128     # 13 t-tiles
SCW = 392          # s-chunk width for score tiles (4 * 392 = 1568)
NSC = S // SCW     # 4
NQC = (S + 127) // 128  # 13 query chunks for ctx (12 full + 1x32)
PAD_REAL = S - (NT - 1) * 128  # 32 real rows in the last t-tile
EPS = 1e-5

_CACHE = {}


def _build_nc(reps=1):
    import concourse.tile as tile
    from concourse import bacc, mybir

    f32 = mybir.dt.float32
    f32r = mybir.dt.float32r
    f16 = mybir.dt.float16
    bf16 = mybir.dt.bfloat16
    AF = mybir.ActivationFunctionType
    ALU = mybir.AluOpType

    nc = bacc.Bacc("TRN2", target_bir_lowering=False, debug=False)

    # xt is the scores-matmul stationary: 16-bit so LDWEIGHTS uses FWL
    # (97ns, hidden under the 163ns MM) instead of the 189ns fp32 path that
    # serialized the scores phase. fp16 keeps 10 mantissa bits.
    xt_d = nc.dram_tensor("xt", [C, SP], f16, kind="ExternalInput").ap()
    gt_d = nc.dram_tensor("gt", [2, C, S], f16, kind="ExternalInput").ap()
    vm_d = nc.dram_tensor("vm", [2, SP, C + 1], bf16, kind="ExternalInput").ap()
    rs_d = nc.dram_tensor("rs", [1, 2], f32, kind="ExternalInput").ap()
    ot_d = nc.dram_tensor("ot", [S, C], f32, kind="ExternalOutput").ap()
    # rows 1536:1568 of the output carry only h0's share in ot; h1's share of
    # the head-packed remainder query chunk (see ctx12) ships here and the
    # host adds them.
    ot2_d = nc.dram_tensor("ot2", [PAD_REAL, C], f32, kind="ExternalOutput").ap()

    with tile.TileContext(nc) as tc, ExitStack() as ctx:
        xw = ctx.enter_context(tc.tile_pool(name="xw", bufs=1))
        sc = ctx.enter_context(tc.tile_pool(name="sc", bufs=1))
        cx = ctx.enter_context(tc.tile_pool(name="cx", bufs=1))
        sm = ctx.enter_context(tc.tile_pool(name="sm", bufs=4))
        # pmm tiles are 2 banks each (see head_scores): 3*2 + 2*1 = 8 banks.
        # bufs=3 lets PE run two scores tiles ahead of the ACT eviction --
        # with bufs=2 every scores tile paid a ~190ns S[act] wait. ctx psum
        # eviction (DVE, ~700ns) is faster than the 13-MM accumulation chain
        # (~1.4us), so pcx bufs=2 suffices.
        pmm = ctx.enter_context(tc.tile_pool(name="pmm", bufs=3, space="PSUM"))
        pcx = ctx.enter_context(tc.tile_pool(name="pcx", bufs=2, space="PSUM"))

        # ---- load inputs once (all persistent across reps) ----
        # The ACT queue must stay DMA-free: its sequencer would issue every
        # queued DMA before reaching the first Exp, stalling the score
        # pipeline on full PSUMs. So the SP and Pool queues carry everything,
        # each in first-use order (first score band's operands land first;
        # vm h0 well before ctx0 at ~17us; gt h1 / vm h1 well before ~35/53us).
        rs_sb = xw.tile([1, 2], f32, tag="rs_sb", name="rs_sb")
        nc.sync.dma_start(rs_sb[:], rs_d[:, :])
        rs_b = xw.tile([128, 2], f32, tag="rs_b", name="rs_b")
        nc.gpsimd.partition_broadcast(rs_b[:], rs_sb[0:1, :])
        # exp scale for the head-packed last key tile: rows 0:32 carry
        # sigma_h0, rows 32:64 sigma_h1, so one ACT evicts both heads.
        rs12 = xw.tile([64, 1], f32, tag="rs12", name="rs12")
        nc.vector.tensor_copy(rs12[0:32, :], rs_b[0:32, 0:1])
        nc.vector.tensor_copy(rs12[32:64, :], rs_b[32:64, 1:2])
        # Full-row homes for the packed key-tile-12 exps (per head, per rep
        # parity). The packed scores MM necessarily lands h1's rows at psum
        # partitions 32:64 (PE column group = output partition group), but
        # the ctx chains need them at 0:32 to pair with vm's partitions and
        # keep full-row stationaries (partial-row LDWEIGHTS can't use the
        # background weight buffer and cost ~+200ns each). A SBUF->SBUF DMA
        # moves them across partitions off the critical path. Rows 32:128
        # are zeroed once: vm's zero pad rows null their contribution, but
        # only if the stationary holds finite values.
        st12f = {(g, p): xw.tile([128, S], bf16, tag=f"st12f{g}{p}",
                                 name=f"st12f{g}{p}")
                 for g in range(2) for p in range(2)}
        for t12 in st12f.values():
            # 32-partition chunks: engine APs from a 32-aligned base may not
            # span more than that quadrant slot
            for p0 in range(PAD_REAL, 128, 32):
                nc.vector.memset(t12[p0:p0 + 32, :], 0.0)
        gts, vms = {}, {}
        for h in range(2):
            gts[h] = [xw.tile([128, S], f16, tag=f"gt{h}{i}", name=f"gt{h}{i}")
                      for i in range(2)]
            vms[h] = [xw.tile([128, C + 1], bf16, tag=f"vm{h}{i}", name=f"vm{h}{i}")
                      for i in range(NT)]
        xt = [xw.tile([128, SP], f16, tag=f"xt{i}", name=f"xt{i}") for i in range(2)]

        def dma_xt(cti, kci):
            kl = slice(kci * 416, (kci + 1) * 416)
            eng = nc.sync if cti == 0 else nc.gpsimd
            eng.dma_start(xt[cti][:, kl], xt_d[cti * 128:(cti + 1) * 128, kl])

        def dma_gt(h, sci, cti, eng):
            sl = slice(sci * SCW, (sci + 1) * SCW)
            eng.dma_start(gts[h][cti][:, sl],
                          gt_d[h, cti * 128:(cti + 1) * 128, sl])

        # SP queue: xt cti0 + gt h0 interleaved by first use, then vm h0, vm h1
        # Pool queue: xt cti1, then gt h1
        dma_xt(0, 0)
        dma_xt(1, 0)
        for cti in range(2):
            dma_gt(0, 0, cti, nc.sync)
        for kci in range(1, 4):
            dma_xt(0, kci)
            dma_xt(1, kci)
        for sci in range(1, NSC):
            for cti in range(2):
                dma_gt(0, sci, cti, nc.sync)
        # vm h0 is needed by ctx0 (~17us): split it across both queues ahead
        # of gt h1, whose deadline (scores1, ~35us) has far more slack
        for ti in range(NT):
            eng = nc.sync if ti % 2 == 0 else nc.gpsimd
            eng.dma_start(vms[0][ti][:], vm_d[0, ti * 128:(ti + 1) * 128, :])
        for sci in range(NSC):
            for cti in range(2):
                dma_gt(1, sci, cti, nc.gpsimd)
        for ti in range(NT):
            nc.sync.dma_start(vms[1][ti][:], vm_d[1, ti * 128:(ti + 1) * 128, :])

        D = {}  # (r, h) -> st tiles ; r -> ctxs tiles

        def head_scores(r, h, filler=None):
            # st[t, s] = exp(sigma_h * scores[s, t]), fused PSUM->SBUF in bf16.
            # Band-pair-major: two 392-wide query bands accumulate into one
            # 2-bank PSUM tile [128, 2, 512] (chunk stride = bank size), so a
            # single ACT instruction evicts 784 columns. That halves the ACT
            # instruction count — the per-instruction 172-cycle PSUM-access
            # bubble made 392-wide evictions (561ns each) the scores-phase
            # rate limiter. ctx chunks still become ready progressively (a
            # band-pair covers 6 of 13 query chunks).
            # Key tile 12 has only 32 real rows, but an MM costs its moving
            # free size regardless of stationary width — so BOTH heads' tile
            # 12 runs head-packed during the h==0 phase (16 MMs in 8 array
            # slots via PE column groups 0/32), then DMAs shift each head's
            # rows into its full-row st12f home for the ctx chains.
            gt = gts[h]
            st = [sc.tile([128, S], bf16, tag=f"st{i}", name=f"st{i}", bufs=2)
                  for i in range(NT - 1)]
            if h == 0:
                st12c = sc.tile([64, S], bf16, tag="st12c", name="st12c", bufs=2)
            for scp in range(NSC // 2):
                for ti in range(NT - 1):
                    tsl = slice(ti * 128, (ti + 1) * 128)
                    ps = pmm.tile([128, 2, 512], f32, tag="ps", name="ps")
                    for j in range(2):
                        sl = slice((2 * scp + j) * SCW, (2 * scp + j + 1) * SCW)
                        for cti in range(2):
                            nc.tensor.matmul(
                                ps[:, j, 0:SCW], xt[cti][:, tsl], gt[cti][:, sl],
                                start=(cti == 0), stop=(cti == 1))
                    nc.scalar.activation(
                        out=st[ti][:, 2 * scp * SCW:(2 * scp + 2) * SCW],
                        in_=ps[:, :, 0:SCW], func=AF.Exp,
                        scale=rs_b[:, h:h + 1])
                    if filler is not None:
                        # ACT paces the scores phase at ~832ns/tile vs 652ns
                        # of scores MMs: weave ~2 ctx MMs into each tile's
                        # slack so PE never waits on the eviction semaphore
                        next(filler, None)
                        next(filler, None)
                if h == 0:
                    ps = pmm.tile([128, 2, 512], f32, tag="ps", name="ps")
                    tsl = slice(1536, 1536 + PAD_REAL)
                    for j in range(2):
                        sl = slice((2 * scp + j) * SCW, (2 * scp + j + 1) * SCW)
                        for cti in range(2):
                            for g in range(2):
                                # per-head accumulation group on its own
                                # partition range; the group checker can't
                                # follow two interleaved groups in one bank
                                nc.tensor.matmul(
                                    ps[32 * g:32 * g + PAD_REAL, j, 0:SCW],
                                    xt[cti][:, tsl], gts[g][cti][:, sl],
                                    start=(cti == 0), stop=(cti == 1),
                                    skip_group_check=True)
                    pair = slice(2 * scp * SCW, (2 * scp + 2) * SCW)
                    nc.scalar.activation(
                        out=st12c[:, pair], in_=ps[0:64, :, 0:SCW],
                        func=AF.Exp, scale=rs12[:, 0:1])
                    # partition-shift to the full-row homes (Pool DMA queue
                    # is idle in steady state; issued a band-pair early so
                    # the latency hides under the remaining scores MMs)
                    nc.gpsimd.dma_start(st12f[0, r % 2][0:PAD_REAL, pair],
                                        st12c[0:PAD_REAL, pair])
                    nc.gpsimd.dma_start(st12f[1, r % 2][0:PAD_REAL, pair],
                                        st12c[32:32 + PAD_REAL, pair])
                    if filler is not None:
                        next(filler, None)
                        next(filler, None)
            D[r, h] = st

        def ctx_steps(r, h):
            # ctx[s, d] += p^T V' / (4 den[s]) ; den rides in column 256.
            # Query chunks 0..11 only: chunk 12 (32 queries) runs head-packed
            # in ctx12 below, so every chain here is a full-width 128.
            # Generator: yields after each PE op so the caller can weave chain
            # MMs between scores tiles as ACT-eviction-wait fill.
            st = D[r, h]
            vm = vms[h]
            if h == 0:
                D[r] = [cx.tile([128, C], f32, tag=f"ctx{i}", name=f"ctx{i}",
                                bufs=1) for i in range(NQC - 1)]
            st12 = st12f[h, r % 2]
            for ci in range(NQC - 1):
                cs = ci * 128
                ps = pcx.tile([128, C + 1], f32, tag="psx", name="psx")
                for ti in range(NT - 1):
                    nc.tensor.matmul(ps[:, :], st[ti][:, cs:cs + 128], vm[ti][:],
                                     start=(ti == 0), stop=False)
                    yield
                nc.tensor.matmul(ps[:, :], st12[:, cs:cs + 128], vm[NT - 1][:],
                                 start=False, stop=True)
                yield
                rec = sm.tile([128, 1], f32, tag="rec", name="rec", bufs=3)
                nc.vector.reciprocal_approx_fast(out=rec[:], in_=ps[:, C:C + 1])
                ctxs = D[r][ci]
                if h == 0:
                    nc.vector.tensor_scalar_mul(ctxs[:, :], ps[:, 0:C], rec[:])
                else:
                    nc.vector.scalar_tensor_tensor(
                        out=ctxs[:, :], in0=ps[:, 0:C], scalar=rec[:],
                        in1=ctxs[:, :], op0=ALU.mult, op1=ALU.add)
                    nc.sync.dma_start(ot_d[cs:cs + 128, :], ctxs[:, :])

        def ctx12(r):
            # Head-packed last query chunk (32 queries per head): h0's chain
            # accumulates into psum partitions 0:32 (PE column group 0), h1's
            # into 32:64 (column group 32). Both chains' stationaries are
            # 32-column slices of full-row st tiles, so their LDWEIGHTS stay
            # background-buffered and consecutive pairs execute concurrently
            # in the array (~2x on this chunk). The group checker can't
            # follow two interleaved accumulation groups in one bank, hence
            # skip_group_check. Reciprocal+scale run once over 64 partitions;
            # h1's rows can't be added to h0's on-device (DVE can't cross
            # partitions), so they ship via ot2 and the host adds.
            st0, st1 = D[r, 0], D[r, 1]
            cs = 1536
            ps = pcx.tile([128, C + 1], f32, tag="psx", name="psx")
            for ti in range(NT - 1):
                nc.tensor.matmul(ps[0:PAD_REAL, :], st0[ti][:, cs:cs + PAD_REAL],
                                 vms[0][ti][:], start=(ti == 0),
                                 stop=False, skip_group_check=True)
                nc.tensor.matmul(ps[32:32 + PAD_REAL, :], st1[ti][:, cs:cs + PAD_REAL],
                                 vms[1][ti][:], start=(ti == 0),
                                 stop=False, skip_group_check=True)
            for g in range(2):
                nc.tensor.matmul(
                    ps[32 * g:32 * g + PAD_REAL, :],
                    st12f[g, r % 2][:, cs:cs + PAD_REAL], vms[g][NT - 1][:],
                    start=False, stop=True, skip_group_check=True)
            rec = sm.tile([128, 1], f32, tag="rec", name="rec", bufs=3)
            nc.vector.reciprocal_approx_fast(out=rec[0:64], in_=ps[0:64, C:C + 1])
            c12 = cx.tile([64, C], f32, tag="c12", name="c12", bufs=1)
            nc.vector.tensor_scalar_mul(c12[:, :], ps[0:64, 0:C], rec[0:64])
            nc.sync.dma_start(ot_d[cs:cs + PAD_REAL, :], c12[0:PAD_REAL, :])
            nc.sync.dma_start(ot2_d[:, :], c12[32:32 + PAD_REAL, :])

        # Software-pipelined rep loop: ctx(h1) of rep r is the weave material
        # for rep r+1's scores(h0) (it reads st1, which isn't rewritten until
        # scores(h1) of r+1, so it can cross the rep boundary freely); ctx(h0)
        # weaves into scores(h1) of the same rep. Leftover chain steps drain
        # as a block right after each scores phase.
        prev = iter(())
        for r in range(reps):
            head_scores(r, 0, filler=prev)
            for _ in prev:
                pass
            if r > 0:
                D.pop((r - 1, 1), None)
                D.pop(r - 1, None)
            f0 = ctx_steps(r, 0)
            head_scores(r, 1, filler=f0)
            for _ in f0:
                pass
            ctx12(r)
            D.pop((r, 0), None)
            prev = ctx_steps(r, 1)
        for _ in prev:
            pass

    nc.finalize()
    return nc


def _get_nc(reps=1):
    key = ("nc", reps)
    if key not in _CACHE:
        _CACHE[key] = _build_nc(reps)
    return _CACHE[key]


def make_in_maps(emb, Wq, Wk, Wv, Wo):
    import ml_dtypes

    emb = np.ascontiguousarray(emb, dtype=np.float32)
    Wq = np.asarray(Wq, np.float64)
    Wk = np.asarray(Wk, np.float64)
    Wv = np.asarray(Wv, np.float64)
    Wo = np.asarray(Wo, np.float64)
    # wg[h] = Wq[h]^T @ Wk[h]  (scores = x wg^T x^T per head)
    wg = np.einsum("hdc,hde->hce", Wq, Wk)
    # wvo[h] = Wv[h]^T @ Wo^T  (folds the output projection into V)
    wvo = np.einsum("hdc,ed->hce", Wv, Wo)
    # closed-form instance-norm stats per (batch, head):
    #   sum(scores)  = u^T G u   (u = column sums of x)
    #   sum(scores^2)= <G, A G A> (A = x^T x)
    # exp scale folds /sqrt(S) and rsqrt(var + eps) into one scalar.
    sigma = np.empty((B, H), np.float32)
    xts, gtb, vmb = [], [], []
    for b in range(B):
        xt = np.zeros((C, SP), np.float16)
        xt[:, :S] = emb[b].transpose(1, 0, 2).reshape(C, S)
        xts.append(xt)
        xb = xt[:, :S].astype(np.float64)
        A = xb @ xb.T
        u = xb.sum(axis=1)
        gth = np.empty((H, C, S), np.float16)
        vmh = np.zeros((H, SP, C + 1), ml_dtypes.bfloat16)
        for h in range(H):
            G = wg[h]
            m_raw = u @ G @ u / (S * S)
            ssq = float(np.sum(G * (A @ G @ A)))
            var_raw = ssq / (S * S) - m_raw * m_raw
            sigma[b, h] = 1.0 / np.sqrt(var_raw + S * EPS)
            gth[h] = G.T @ xb                       # [C, S]
            vmh[h, :S, :C] = (xb.T @ wvo[h]).astype(ml_dtypes.bfloat16)
            vmh[h, :S, C] = ml_dtypes.bfloat16(float(H))
        gtb.append(gth)
        vmb.append(vmh)
    in_maps = []
    for core in range(8):
        b, g = core // 2, core % 2
        hs = [2 * g, 2 * g + 1]
        in_maps.append({
            "xt": xts[b],
            "gt": np.ascontiguousarray(gtb[b][hs]),
            "vm": np.ascontiguousarray(vmb[b][hs]),
            "rs": sigma[b, hs].reshape(1, 2),
        })
    return in_maps


def gather_out(results):
    out = np.empty((B, S, C), np.float32)
    for b in range(B):
        out[b] = results[2 * b]["ot"] + results[2 * b + 1]["ot"]
        # rows 1536:1568 carry only h0's share in ot; h1's share of the
        # head-packed remainder chunk arrives separately in ot2
        out[b, 1536:1536 + PAD_REAL] += (results[2 * b]["ot2"]
                                         + results[2 * b + 1]["ot2"])
    return out.reshape(B, T, C, N)


def _get_runner():
    """Cached PJRT executable: run_bass_kernel_spmd re-jits per call, which
    costs seconds of XLA compile on every invocation; build the sharded
    callable once and reuse it."""
    if "runner" in _CACHE:
        return _CACHE["runner"]
    import jax
    from jax.sharding import Mesh, PartitionSpec, NamedSharding
    from jax.experimental.shard_map import shard_map
    from concourse import mybir
    from concourse.bass2jax import (_bass_exec_p, install_neuronx_cc_hook,
                                    partition_id_tensor)

    install_neuronx_cc_hook()
    nc = _get_nc()
    in_names, out_names, out_avals, zero_shapes = [], [], [], []
    partition_name = nc.partition_id_tensor.name if nc.partition_id_tensor else None
    for alloc in nc.m.functions[0].allocations:
        if not isinstance(alloc, mybir.MemoryLocationSet):
            continue
        name = alloc.memorylocations[0].name
        if alloc.kind == "ExternalInput":
            if name != partition_name:
                in_names.append(name)
        elif alloc.kind == "ExternalOutput":
            shape = tuple(alloc.tensor_shape)
            dtype = mybir.dt.np(alloc.dtype)
            out_names.append(name)
            out_avals.append(jax.core.ShapedArray(shape, dtype))
            zero_shapes.append((shape, dtype))
    n_params = len(in_names)
    all_in = list(in_names) + list(out_names)
    if partition_name is not None:
        all_in.append(partition_name)

    def _body(*args):
        operands = list(args)
        if partition_name is not None:
            operands.append(partition_id_tensor())
        return tuple(_bass_exec_p.bind(
            *operands, out_avals=tuple(out_avals), in_names=tuple(all_in),
            out_names=tuple(out_names), lowering_input_output_aliases=(),
            sim_require_finite=True, sim_require_nnan=True, nc=nc))

    n_cores = 8
    mesh = Mesh(np.asarray(jax.devices()[:n_cores]), ("core",))
    sharded = jax.jit(
        shard_map(_body, mesh=mesh,
                  in_specs=(PartitionSpec("core"),) * (n_params + len(out_names)),
                  out_specs=(PartitionSpec("core"),) * len(out_names),
                  check_rep=False),
        keep_unused=True)

    def run(in_maps):
        per_core = [[np.asarray(m[nm]) for nm in in_names] for m in in_maps]
        concat_in = [np.concatenate([per_core[c][i] for c in range(n_cores)], axis=0)
                     for i in range(n_params)]
        concat_zeros = [np.zeros((n_cores * s[0], *s[1:]), d)
                        for (s, d) in zero_shapes]
        outs = sharded(*concat_in, *concat_zeros)
        return [{out_names[i]: np.asarray(outs[i]).reshape(
                     n_cores, *out_avals[i].shape)[c]
                 for i in range(len(out_names))} for c in range(n_cores)]

    _CACHE["runner"] = run
    return run


def kernel(emb, Wq, Wk, Wv, Wo):
    in_maps = make_in_maps(emb, Wq, Wk, Wv, Wo)
    try:
        return gather_out(_get_runner()(in_maps))
    except Exception:
        from concourse.bass_utils import run_bass_kernel_spmd
        nc = _get_nc()
        res = run_bass_kernel_spmd(nc, in_maps, list(range(8)))
        return gather_out(res.results)

